# revision 13
# baseline (speedup 1.0000x reference)
"""CTC loss (keras ctc_batch_cost semantics, full lengths) on 8 Trainium2 cores.

Strategy (data parallel, B=512 -> 64 samples/core):
- Exp-space DP with periodic max-rescaling instead of log-space logsumexp.
- States split even/odd: E[j]=alpha[2j] (129 blanks), O[j]=alpha[2j+1] (128
  labels); O kept in a 129-wide tile with col0==0 so O[j-1] is a shifted slice.
- Partitions 0-63 run the forward DP (t=0..255); partitions 64-127 run the
  backward DP (t=511..256) in reversed state order, which has the *identical*
  recurrence -> one instruction stream drives both. 256 unified steps, then
  a small combine: loss = -(log(dot(alpha_255, beta_255)) + accF + accB).
- DP state in bf16 (TT ops hit the DVE 2x_1p mode); gathered probs stay fp32.
- Triangular trim: after step k only states j <= k are live, so ops narrow
  to min(k+2, L) columns for the first half of the DP.
- Label-prob gather: per-sample label indices are baked into uint16 index
  tables (host prep). GPSIMD indirect_copy gathers with an "octet" layout
  (16 partitions = one sample's t-chunks share the index stream), then ONE
  SBUF->SBUF DMA per call repacks into the sample-major DP layout.
- Window loads are interleaved with the DP so window w+1 streams in while
  the DP consumes window w.
"""

import numpy as np
import ml_dtypes

import concourse.bass as bass
import concourse.bacc as bacc
import concourse.tile as tile
from concourse import mybir
from concourse._compat import get_trn_type
from concourse.bass_utils import run_bass_kernel_spmd

F32 = mybir.dt.float32
BF16 = mybir.dt.bfloat16
U16 = mybir.dt.uint16
ALU = mybir.AluOpType
AF = mybir.ActivationFunctionType
AX = mybir.AxisListType

B, T, C, L = 512, 512, 100, 128
BLANK = C - 1
EPS = 1e-7
NCORES = 8
BPC = B // NCORES          # 64 samples per core
NW = 4                     # windows over the 256 unified steps
WSLOTS = 256 // NW         # 64 slots per window
WP = WSLOTS // 16          # 4 t-rows per partition per octet call
NCALL = 16                 # octet calls per window (8 fwd + 8 bwd)
NIDX = WP * (L + 1)        # 516 gather indices per group per call
IDXC = ((NIDX + 15) // 16 + 1) // 2 * 2  # idx cols per call, even for 4B-aligned slices
RESC = 64                  # rescale cadence
RREV = 132                 # reversal gather entries (mult of 4; 129 used)
RIDXC = ((RREV + 15) // 16 + 1) // 2 * 2   # reversal idx cols (even)


# ----------------------------------------------------------------- host prep
def _host_tables(y_true_core):
    """Index/mask tables from labels. y_true_core: (64, L) int."""
    lab = y_true_core.astype(np.int64)
    lrev = lab[:, ::-1]
    mF = np.zeros((BPC, L), np.float32)
    mF[:, 1:] = (lab[:, 1:] != lab[:, :-1]).astype(np.float32)
    mB = np.zeros((BPC, L), np.float32)
    mB[:, 1:] = mF[:, ::-1][:, :-1]
    mconst = np.concatenate([mF, mB], axis=0)            # (128, L)
    mshift = np.zeros_like(mconst)                       # Om mask: m[j+1]
    mshift[:, : L - 1] = mconst[:, 1:]
    mshift = mshift.astype(ml_dtypes.bfloat16)           # 0/1: exact in bf16
    mcomb = np.zeros((128, L), np.float32)
    mcomb[0:64, : L - 1] = mF[:, 1:]                     # combine: mF_ext[j+1]

    # gather index tables: 16 calls x (128, IDXC) packed as (128, 16*IDXC)
    gidx = np.zeros((128, NCALL * IDXC), np.uint16)
    for o in range(NCALL):
        fwd = o < 8
        for g in range(8):
            s = 8 * o + g if fwd else 8 * (o - 8) + g
            labs = lab[s] if fwd else lrev[s]
            stream = np.empty(NIDX, np.uint16)
            for wl in range(WP):
                q = wl if fwd else (WP - 1 - wl)
                stream[wl * (L + 1): wl * (L + 1) + L] = q * C + labs
                stream[wl * (L + 1) + L] = q * C + BLANK
            for i in range(NIDX):
                gidx[16 * g + i % 16, o * IDXC + i // 16] = stream[i]

    # reversal indices (same stream for every 16-partition group): j -> 128-j,
    # padded to RREV=132 entries (multiple of 4 for the gpsimd gather ucode)
    ridx = np.zeros((128, RIDXC), np.uint16)
    for g in range(8):
        for i in range(RREV):
            ridx[16 * g + i % 16, i // 16] = max(L - i, 0)
    return gidx, ridx, mshift, mcomb


# ------------------------------------------------------------- bass program
_PROGRAM = None


def _build_program(snap_ks=(), snap_gwin=False, nsteps=256, null=False,
                   reps=1):
    if null:
        nc = bacc.Bacc(get_trn_type() or "TRN2", target_bir_lowering=False,
                       debug=False, enable_asserts=False)
        loss_d = nc.dram_tensor("loss", [BPC, 1], F32, kind="ExternalOutput").ap()
        with tile.TileContext(nc) as tc:
            with tc.tile_pool(name="p", bufs=1) as pool:
                t = pool.tile([BPC, 1], F32, name="nullt")
                nc.vector.memset(t[:], 0.0)
                nc.sync.dma_start(loss_d[:], t[:])
        nc.compile()
        return nc
    nc = bacc.Bacc(get_trn_type() or "TRN2", target_bir_lowering=False,
                   debug=False, enable_asserts=False)
    snaps = {}
    if snap_ks or snap_gwin:
        for nm, w in (("WEs", RREV), ("WOxs", RREV), ("RWE", RREV),
                      ("RWOx", RREV), ("betaE", L + 1), ("betaO", L),
                      ("dE", 1), ("dO", 1), ("ds", 1), ("accB", 1), ("lg2", 1)):
            p = 64
            snaps[f"snapC_{nm}"] = nc.dram_tensor(
                f"snapC_{nm}", [p, w], F32, kind="ExternalOutput").ap()
        for k in snap_ks:
            for nm in ("E", "Ox", "Om", "acc"):
                w = 1 if nm == "acc" else L + 1
                dt = F32 if nm == "acc" else BF16
                snaps[f"snap{nm}_{k}"] = nc.dram_tensor(
                    f"snap{nm}_{k}", [128, w], dt, kind="ExternalOutput").ap()
        if snap_gwin:
            for w in range(NW):
                snaps[f"snapgw_{w}"] = nc.dram_tensor(
                    f"snapgw_{w}", [128, WSLOTS * (L + 1)], F32,
                    kind="ExternalOutput").ap()

    yp = nc.dram_tensor("yp", [BPC, T, C], F32, kind="ExternalInput").ap()
    gidx_d = nc.dram_tensor("gidx", [128, NCALL * IDXC], U16,
                            kind="ExternalInput").ap()
    ridx_d = nc.dram_tensor("ridx", [128, RIDXC], U16,
                            kind="ExternalInput").ap()
    mshift_d = nc.dram_tensor("mshift", [128, L], BF16,
                              kind="ExternalInput").ap()
    mcomb_d = nc.dram_tensor("mcomb", [128, L], F32,
                             kind="ExternalInput").ap()
    loss_d = nc.dram_tensor("loss", [BPC, 1], F32, kind="ExternalOutput").ap()

    with tile.TileContext(nc) as tc:
        with (
            tc.tile_pool(name="consts", bufs=1) as consts,
            tc.tile_pool(name="raw", bufs=4) as rawp,
            tc.tile_pool(name="gout", bufs=4) as goutp,
            tc.tile_pool(name="gwin", bufs=2) as gwinp,
            tc.tile_pool(name="pbe", bufs=2) as pbep,
            tc.tile_pool(name="state", bufs=1) as statep,
            tc.tile_pool(name="temps", bufs=3) as tmpp,
            tc.tile_pool(name="small", bufs=2) as smallp,
        ):
            # constants
            gidx_s = consts.tile([128, NCALL * IDXC], U16, tag="gidx")
            ridx_s = consts.tile([128, RIDXC], U16, tag="ridx")
            msh = consts.tile([128, L], BF16, tag="msh")
            mcb = consts.tile([128, L], F32, tag="mcb")
            nc.sync.dma_start(gidx_s[:], gidx_d[:])
            nc.sync.dma_start(ridx_s[:], ridx_d[:])
            nc.sync.dma_start(msh[:], mshift_d[:])
            nc.sync.dma_start(mcb[:], mcomb_d[:])

            # persistent state (ping-pong), bf16: TT ops run in 2x_1p mode
            Es = [statep.tile([128, L + 1], BF16, name=f"E{i}", tag=f"E{i}")
                  for i in range(2)]
            Oxs = [statep.tile([128, L + 1], BF16, name=f"Ox{i}", tag=f"Ox{i}")
                   for i in range(2)]
            Oms = [statep.tile([128, L + 1], BF16, name=f"Om{i}", tag=f"Om{i}")
                   for i in range(2)]
            acc = statep.tile([128, 1], F32, tag="acc")

            def body():
                for tile_ in (Es[0], Es[1], Oxs[0], Oxs[1], Oms[0], Oms[1]):
                    nc.vector.memset(tile_[:], 0.0)
                nc.vector.memset(acc[:], 0.0)
                nc.vector.memset(Es[0][:, 0:1], 1.0)  # delta init

                # window load: raw DMA + gather + fused repack. Issued
                # per-window so window w+1's DMAs/gathers run while the DP
                # chews window w.
                def load_window(w):
                    gwin = gwinp.tile([128, WSLOTS * (L + 1)], F32,
                                      name="gwin", tag="gwin")
                    for o in range(NCALL):
                        raw = rawp.tile([128, WP * 16 * C // 16], F32,
                                        name="raw", tag="raw")
                        if o < 8:
                            s0 = 8 * o
                            src = (yp[s0:s0 + 8,
                                      w * WSLOTS: (w + 1) * WSLOTS, :]
                                   .rearrange("s (r q) c -> s r (q c)", r=16))
                        else:
                            s0 = 8 * (o - 8)
                            # bwd window w covers t in [512-64(w+1), 511-64w];
                            # row r gets the (15-r)-th ascending 4-run
                            t_lo = 512 - (w + 1) * WSLOTS
                            src = (yp[s0:s0 + 8, t_lo: t_lo + WSLOTS, :]
                                   .rearrange("s (r q) c -> s r (q c)", r=16)
                                   [:, ::-1, :])
                        nc.sync.dma_start(raw[:], src)
                        gout = goutp.tile([128, NIDX], F32, name="gout",
                                          tag="gout")
                        nc.gpsimd.indirect_copy(
                            gout[:], raw[:],
                            gidx_s[:, o * IDXC:(o + 1) * IDXC], True)
                        # repack all 8 octets in ONE dma: flattened rasters
                        # line up (gout p-major 128x516 -> 8 gwin rows)
                        r0 = 8 * o if o < 8 else 64 + 8 * (o - 8)
                        nc.sync.dma_start(gwin[r0:r0 + 8, :], gout[:, :])
                    return gwin

                # unified DP: 256 steps. Reachability: after step k only
                # states j <= k are nonzero (fwd and bwd alike), so narrow
                # every op to wv = min(k+2, L) columns; unwritten tail
                # columns stay zero from the memset and widths only grow.
                cur = 0
                nw_used = (nsteps + WSLOTS - 1) // WSLOTS
                gwin_next = load_window(0) if nw_used else None
                for w in range(nw_used):
                    gwin = gwin_next
                    if w + 1 < nw_used:
                        gwin_next = load_window(w + 1)
                    pbe = pbep.tile([128, WSLOTS], F32, name="pbe", tag="pbe")
                    nc.vector.tensor_scalar_add(
                        pbe[:], gwin[:, L::L + 1], float(EPS))
                    if snap_gwin:
                        nc.sync.dma_start(snaps[f"snapgw_{w}"][:], gwin[:])

                    for slot in range(min(WSLOTS, nsteps - w * WSLOTS)):
                        k = w * WSLOTS + slot
                        E0, Ox0, Om0 = Es[cur], Oxs[cur], Oms[cur]
                        E1, Ox1, Om1 = (Es[1 - cur], Oxs[1 - cur],
                                        Oms[1 - cur])
                        wv = min(k + 2, L)
                        wu = min(k + 2, L + 1)
                        gO = gwin[:, slot * (L + 1): slot * (L + 1) + wv]

                        t1 = tmpp.tile([128, L], BF16, name="t1", tag="t1")
                        t2 = tmpp.tile([128, L], BF16, name="t2", tag="t2")
                        u = tmpp.tile([128, L + 1], BF16, name="u", tag="u")
                        nc.vector.tensor_add(t1[:, :wv], Ox0[:, 1:1 + wv],
                                             E0[:, :wv])
                        nc.vector.tensor_add(t2[:, :wv], t1[:, :wv],
                                             Om0[:, :wv])
                        nc.vector.scalar_tensor_tensor(
                            out=Ox1[:, 1:1 + wv], in0=gO, scalar=float(EPS),
                            in1=t2[:, :wv], op0=ALU.add, op1=ALU.mult)
                        nc.vector.tensor_mul(Om1[:, 1:1 + wv],
                                             Ox1[:, 1:1 + wv], msh[:, :wv])
                        nc.vector.tensor_add(u[:, :wu], E0[:, :wu],
                                             Ox0[:, :wu])
                        nc.scalar.mul(E1[:, :wu], u[:, :wu],
                                      pbe[:, slot:slot + 1])
                        cur = 1 - cur

                        if (k + 1) % RESC == 0:
                            Ec, Oxc, Omc = Es[cur], Oxs[cur], Oms[cur]
                            r1 = smallp.tile([128, 1], F32, name="r1",
                                             tag="r1")
                            r2 = smallp.tile([128, 1], F32, name="r2",
                                             tag="r2")
                            rm = smallp.tile([128, 1], F32, name="rm",
                                             tag="rm")
                            ri = smallp.tile([128, 1], F32, name="ri",
                                             tag="ri")
                            lg = smallp.tile([128, 1], F32, name="lg",
                                             tag="lg")
                            nc.vector.tensor_reduce(r1[:], Ec[:], axis=AX.X,
                                                    op=ALU.max)
                            nc.vector.tensor_reduce(r2[:], Oxc[:], axis=AX.X,
                                                    op=ALU.max)
                            nc.vector.tensor_tensor(rm[:], r1[:], r2[:],
                                                    ALU.max)
                            nc.vector.reciprocal(ri[:], rm[:])
                            nc.vector.tensor_scalar_mul(Ec[:], Ec[:], ri[:])
                            nc.vector.tensor_scalar_mul(Oxc[:], Oxc[:], ri[:])
                            nc.vector.tensor_scalar_mul(Omc[:], Omc[:], ri[:])
                            nc.scalar.activation(lg[:], ri[:], AF.Ln)
                            nc.vector.tensor_sub(acc[:], acc[:], lg[:])

                        if k in snap_ks:
                            nc.sync.dma_start(snaps[f"snapE_{k}"][:],
                                              Es[cur][:])
                            nc.sync.dma_start(snaps[f"snapOx_{k}"][:],
                                              Oxs[cur][:])
                            nc.sync.dma_start(snaps[f"snapOm_{k}"][:],
                                              Oms[cur][:])
                            nc.sync.dma_start(snaps[f"snapacc_{k}"][:],
                                              acc[:])

                # combine (fp32): cast the bf16 finals up front
                EfB, OxfB = Es[cur], Oxs[cur]
                Ef = statep.tile([128, L + 1], F32, tag="EfF")
                Oxf = statep.tile([128, L + 1], F32, tag="OxfF")
                nc.vector.tensor_copy(Ef[:], EfB[:])
                nc.vector.tensor_copy(Oxf[:], OxfB[:])
                WEs = statep.tile([128, RREV], F32, tag="WEs")
                WOxs = statep.tile([128, RREV], F32, tag="WOxs")
                accB = statep.tile([64, 1], F32, tag="accB")
                RWE = statep.tile([128, RREV], F32, tag="RWE")
                RWOx = statep.tile([128, RREV], F32, tag="RWOx")
                nc.vector.memset(WEs[:], 0.0)
                nc.vector.memset(WOxs[:], 0.0)
                nc.sync.dma_start(WEs[0:64, 0:L + 1], Ef[64:128, :])
                nc.sync.dma_start(WOxs[0:64, 0:L + 1], Oxf[64:128, :])
                nc.sync.dma_start(accB[:], acc[64:128, :])
                nc.gpsimd.indirect_copy(RWE[:], WEs[:], ridx_s[:], True)
                nc.gpsimd.indirect_copy(RWOx[:], WOxs[:], ridx_s[:], True)

                betaE = statep.tile([64, L + 1], F32, tag="betaE")
                tb1 = statep.tile([64, L], F32, tag="tb1")
                tb2 = statep.tile([64, L], F32, tag="tb2")
                betaO = statep.tile([64, L], F32, tag="betaO")
                junkE = statep.tile([64, L + 1], F32, tag="junkE")
                junkO = statep.tile([64, L], F32, tag="junkO")
                dE = statep.tile([64, 1], F32, tag="dE")
                dO = statep.tile([64, 1], F32, tag="dO")
                ds = statep.tile([64, 1], F32, tag="ds")
                lg2 = statep.tile([64, 1], F32, tag="lg2")
                lnS = statep.tile([64, 1], F32, tag="lnS")
                tot = statep.tile([64, 1], F32, tag="tot")
                tot2 = statep.tile([64, 1], F32, tag="tot2")
                res = statep.tile([64, 1], F32, tag="res")

                nc.vector.tensor_add(betaE[:], RWE[0:64, 0:L + 1],
                                     RWOx[0:64, 0:L + 1])
                nc.vector.tensor_mul(tb1[:], mcb[0:64, :],
                                     RWOx[0:64, 1:L + 1])
                nc.vector.tensor_add(tb2[:], RWE[0:64, 1:L + 1], tb1[:])
                nc.vector.tensor_add(betaO[:], RWOx[0:64, 0:L], tb2[:])
                nc.vector.scalar_tensor_tensor(
                    out=junkE[:], in0=Ef[0:64, :], scalar=1.0, in1=betaE[:],
                    op0=ALU.mult, op1=ALU.mult, accum_out=dE[:])
                nc.vector.scalar_tensor_tensor(
                    out=junkO[:], in0=Oxf[0:64, 1:], scalar=1.0, in1=betaO[:],
                    op0=ALU.mult, op1=ALU.mult, accum_out=dO[:])
                nc.vector.tensor_add(ds[:], dE[:], dO[:])
                # ds can be far below 2^-64 (outside the ACT Ln LUT range), so
                # ln(ds) = 2*ln(sqrt(ds*2^20)) - 20*ln2 keeps the LUT in range.
                nc.scalar.activation(lg2[:], ds[:], AF.Sqrt,
                                     scale=float(2.0 ** 20))
                nc.scalar.activation(lnS[:], lg2[:], AF.Ln)
                nc.vector.tensor_add(tot[:], acc[0:64, :], accB[:])
                nc.vector.tensor_scalar_add(tot2[:], tot[:],
                                            float(-20.0 * np.log(2.0)))
                nc.vector.scalar_tensor_tensor(
                    out=res[:], in0=lnS[:], scalar=-2.0, in1=tot2[:],
                    op0=ALU.mult, op1=ALU.subtract)
                nc.sync.dma_start(loss_d[:], res[:])
                if snap_ks or snap_gwin:
                    for nm, t in (("WEs", WEs), ("WOxs", WOxs), ("RWE", RWE),
                                  ("RWOx", RWOx), ("betaE", betaE),
                                  ("betaO", betaO), ("dE", dE), ("dO", dO),
                                  ("ds", ds), ("accB", accB), ("lg2", lg2)):
                        nc.sync.dma_start(snaps[f"snapC_{nm}"][:], t[0:64, :])

            if reps > 1:
                # hardware rep-loop for wall-clock timing: state re-init and
                # the full pipeline run inside the loop.
                with tc.For_i(0, reps, 1):
                    body()
            else:
                body()

    nc.compile()
    return nc


def _get_program():
    global _PROGRAM
    if _PROGRAM is None:
        _PROGRAM = _build_program()
    return _PROGRAM


def make_in_maps(y_true, y_pred):
    y_true = np.asarray(y_true)
    y_pred = np.ascontiguousarray(np.asarray(y_pred, dtype=np.float32))
    in_maps = []
    for c in range(NCORES):
        sl = slice(c * BPC, (c + 1) * BPC)
        gidx, ridx, mshift, mcomb = _host_tables(y_true[sl])
        in_maps.append({
            "yp": y_pred[sl],
            "gidx": gidx,
            "ridx": ridx,
            "mshift": mshift,
            "mcomb": mcomb,
        })
    return in_maps


def kernel(y_true, y_pred):
    nc = _get_program()
    in_maps = make_in_maps(y_true, y_pred)
    res = run_bass_kernel_spmd(nc, in_maps, core_ids=list(range(NCORES)))
    out = np.concatenate([res.results[c]["loss"] for c in range(NCORES)], axis=0)
    return out.astype(np.float32)


if __name__ == "__main__":
    y_true = np.load("y_true.npy")
    y_pred = np.load("y_pred.npy")
    out = kernel(y_true, y_pred)
    exp = np.load("expected_np.npy")
    err = np.abs(out.ravel() - exp) / np.maximum(1.0, np.abs(exp))
    print("kernel out[:4]:", out.ravel()[:4])
    print("expected [:4]:", exp[:4])
    print("max rel err:", err.max())
